# revision 12
# baseline (speedup 1.0000x reference)
"""Trainium2 Bass kernel for EuclideanCodebook (VQ) forward.

Problem: x [16, 4096, 256] f32, embed [2048, 256] f32.
  dist[t, k] = -(||x_t||^2 - 2 x_t.e_k + ||e_k||^2)
  ind[t]     = argmax_k dist  (== argmax_k 2 x_t.e_k - ||e_k||^2)
  quantize   = embed[ind]

Sharding: data-parallel over the flattened token axis (65536 tokens ->
8192/core on 8 cores), codebook replicated.

Per-core pipeline (128-token tiles, 64 tiles):
  PE   : score = x @ (2 embed).T  via fp32 matmuls into PSUM
  DVE  : dist = psum + (-||e||^2)   (tensor_tensor add, PSUM+SBUF->SBUF)
         max -> top-8 values; max_index -> first-occurrence argmax (exact
         jnp.argmax tie semantics)
  DMA  : gpsimd indirect gather embed[idx] -> SBUF, HWDGE write to DRAM out

  (tensor_tensor_reduce and gpsimd scalar_tensor_tensor crash this HW; the
  fp32r (TF32) matmul path is numerically unusable for exact argmax.)

Host side: x is pre-transposed per core ([256, 8192]) so the contraction
dim lands on SBUF partitions without on-device transposes.
"""

import numpy as np

import concourse.bass as bass
import concourse.mybir as mybir
import concourse.tile as tile
from concourse import bacc
from concourse.bass_utils import run_bass_kernel_spmd

P = 128
D = 256
K = 2048
KC = 512  # k chunk (one PSUM bank of fp32)
NCORES = 8

F32 = mybir.dt.float32
F32R = mybir.dt.float32r
I32 = mybir.dt.int32


def build_nc(ntok: int, group: int = 8):
    """Build the per-core Bass program for `ntok` tokens (multiple of P*group)."""
    nt = ntok // P
    assert nt % group == 0
    ng = nt // group

    nc = bacc.Bacc("TRN2", target_bir_lowering=False, debug=False,
                   num_devices=NCORES)

    xt_d = nc.dram_tensor("xT", [D, ntok], F32, kind="ExternalInput").ap()
    emb2_d = nc.dram_tensor("embT2", [D, K], F32, kind="ExternalInput").ap()
    negb_d = nc.dram_tensor("negb", [P, K], F32, kind="ExternalInput").ap()
    embed_d = nc.dram_tensor("embed", [K, D], F32, kind="ExternalInput").ap()

    q_d = nc.dram_tensor("quantize", [ntok, D], F32, kind="ExternalOutput").ap()
    ind_d = nc.dram_tensor("eind", [nt, P], I32, kind="ExternalOutput").ap()

    from contextlib import ExitStack
    with tile.TileContext(nc) as tc, ExitStack() as ctx:
        const_pool = ctx.enter_context(tc.tile_pool(name="const", bufs=1))
        xg_pool = ctx.enter_context(tc.tile_pool(name="xg", bufs=3))
        psum_pool = ctx.enter_context(tc.tile_pool(name="psum", bufs=2, space="PSUM"))
        dist_pool = ctx.enter_context(tc.tile_pool(name="dist", bufs=3))
        m_pool = ctx.enter_context(tc.tile_pool(name="m", bufs=3))
        q_pool = ctx.enter_context(tc.tile_pool(name="q", bufs=2))

        # one-time constant loads
        emb_sb = []
        for d in range(2):
            t = const_pool.tile([P, K], F32, tag=f"emb{d}")
            nc.sync.dma_start(t[:], emb2_d[d * P:(d + 1) * P, :])
            emb_sb.append(t)
        negb_sb = const_pool.tile([P, K], F32, tag="negb")
        nc.sync.dma_start(negb_sb[:], negb_d[:])
        idx_all = const_pool.tile([P, nt], I32, tag="idxall")

        for g in range(ng):
            xg = []
            for d in range(2):
                t = xg_pool.tile([P, group * P], F32, tag=f"xg{d}")
                nc.sync.dma_start(
                    t[:], xt_d[d * P:(d + 1) * P,
                               g * group * P:(g + 1) * group * P])
                xg.append(t)
            for i in range(group):
                t = g * group + i
                ps = psum_pool.tile([P, K], F32, tag="ps")
                for kc in range(K // KC):
                    sl = slice(kc * KC, (kc + 1) * KC)
                    nc.tensor.matmul(ps[:, sl],
                                     lhsT=xg[0][:, i * P:(i + 1) * P],
                                     rhs=emb_sb[0][:, sl],
                                     start=True, stop=False)
                    nc.tensor.matmul(ps[:, sl],
                                     lhsT=xg[1][:, i * P:(i + 1) * P],
                                     rhs=emb_sb[1][:, sl],
                                     start=False, stop=True)
                # dist = psum + (-||e||^2), then top-8 + first-occurrence index
                dist = dist_pool.tile([P, K], F32, tag="dist")
                nc.vector.tensor_tensor(out=dist[:], in0=ps[:], in1=negb_sb[:],
                                        op=mybir.AluOpType.add)
                m8 = m_pool.tile([P, 8], F32, tag="m8")
                nc.vector.max(out=m8[:], in_=dist[:])
                i8 = m_pool.tile([P, 8], mybir.dt.uint32, tag="i8")
                nc.vector.max_index(out=i8[:], in_max=m8[:], in_values=dist[:])
                nc.vector.tensor_copy(idx_all[:, t:t + 1],
                                      i8[:, 0:1].bitcast(I32))

                # gather embed rows -> quantize (single-offset indirect DMA;
                # multi-offset gathers return garbage on HW)
                qt = q_pool.tile([P, D], F32, tag="qt")
                nc.gpsimd.indirect_dma_start(
                    out=qt[:],
                    out_offset=None,
                    in_=embed_d[:],
                    in_offset=bass.IndirectOffsetOnAxis(
                        ap=idx_all[:, t:t + 1], axis=0),
                )
                nc.sync.dma_start(q_d[t * P:(t + 1) * P, :], qt[:])

        nc.sync.dma_start(ind_d.rearrange("t p -> p t"), idx_all[:])

    nc.compile()
    return nc


def make_host_inputs(x_core: np.ndarray, embed: np.ndarray):
    """Per-core input map. x_core [ntok, D] f32, embed [K, D] f32."""
    xt = np.ascontiguousarray(x_core.T)  # [D, ntok]
    emb2 = np.ascontiguousarray((embed * np.float32(2.0)).T)  # [D, K]
    bneg = (-(embed.astype(np.float64) ** 2).sum(1)).astype(np.float32)
    negb = np.broadcast_to(bneg, (P, K))
    return {
        "xT": xt,
        "embT2": emb2,
        "negb": np.ascontiguousarray(negb),
        "embed": np.ascontiguousarray(embed.astype(np.float32)),
    }


_CACHE = {}


def _get_nc(ntok):
    if ntok not in _CACHE:
        _CACHE[ntok] = build_nc(ntok)
    return _CACHE[ntok]


def kernel(x: np.ndarray, embed: np.ndarray):
    x = np.asarray(x, dtype=np.float32)
    embed = np.asarray(embed, dtype=np.float32)
    shape = x.shape
    xf = x.reshape(-1, shape[-1])
    n = xf.shape[0]
    assert n % NCORES == 0
    npc = n // NCORES

    nc = _get_nc(npc)
    in_maps = [make_host_inputs(xf[c * npc:(c + 1) * npc], embed)
               for c in range(NCORES)]
    res = run_bass_kernel_spmd(nc, in_maps, list(range(NCORES))).results

    q = np.concatenate([res[c]["quantize"] for c in range(NCORES)], axis=0)
    ind = np.concatenate([res[c]["eind"].reshape(-1) for c in range(NCORES)])
    quantize = q.reshape(*shape)
    embed_ind = ind.astype(np.int32).reshape(*shape[:-1])
    return quantize, embed_ind
